# revision 2
# baseline (speedup 1.0000x reference)
"""SpMM (COO segment-sum) kernel for trn2, 8 NeuronCores.

out[i] = sum_{e: row[e]==i} val[e] * x[col[e]]   (N=65536, E~1M, D=64)

Strategy (dest-row 1D sharding, per spec hint):
- Host: stable-sort edges by destination row; shard rows 8192/core; within a
  core, bucket edges into 64-row windows, split each window into two column
  streams (col<32768 / col>=32768, so node indices fit int16 for dma_gather);
  pad each (window, stream) bucket to whole 128-edge blocks (pad: idx=0,
  val=0). Block counts per bucket are maxed across cores so all 8 cores run
  one SPMD program.
- Device, per 128-edge block:
    g[p, :]   = x[colidx[p]]                      (dma_gather, 4 SWDGE queues)
    sel[p, r] = val[p] * (r == row_local[p])      (DVE tensor_scalar fused)
    psum[r, f] += sum_p sel[p, r] * g[p, f]       (PE matmul, PSUM window acc)
  Windows drain PSUM -> SBUF (ACT copy), batched DMA to out.
"""

import os
import numpy as np

N_NODES = 65536
D = 64
P = 128
N_CORES = 8
ROWS_PER_CORE = N_NODES // N_CORES   # 8192
W = 64                               # rows per PSUM window
WINDOWS = ROWS_PER_CORE // W         # 128
HALF = N_NODES // 2                  # int16-addressable half
CHUNK_BLOCKS = 8                     # <=1024 idxs per dma_gather
NQ = 4                               # SWDGE queues
OUT_BATCH = 16                       # windows per output DMA

LAST_EXEC_NS = None


def _pack(row, col, val):
    """Host-side packing. Returns per-core device arrays + shared block map."""
    E = row.shape[0]
    core = row // ROWS_PER_CORE
    win = (row % ROWS_PER_CORE) // W
    strm = (col >= HALF).astype(np.int64)

    # group key in program order: (core major for split, then w, s)
    gkey = (win * 2 + strm).astype(np.int64)
    order = np.lexsort((row, gkey, core))  # sort by core, then group, then row
    rs, cs, vs, gs, cos = (row[order], col[order], val[order],
                           gkey[order], core[order])

    cnt = np.zeros((N_CORES, WINDOWS * 2), np.int64)
    np.add.at(cnt, (cos, gs), 1)
    B = -(-cnt // P)                      # ceil blocks per (core, group)
    B = B.max(axis=0)                     # [WINDOWS*2] shared across cores
    for w in range(WINDOWS):              # every window needs >=1 block
        if B[2 * w] + B[2 * w + 1] == 0:
            B[2 * w] = 1
    group_base = np.zeros(WINDOWS * 2 + 1, np.int64)
    np.cumsum(B * P, out=group_base[1:])
    total_blocks = int(B.sum())
    slots = total_blocks * P

    # per-edge slot position: group base + rank within (core, group)
    ckey = cos * (WINDOWS * 2) + gs
    starts = np.zeros(E, np.int64)
    newgrp = np.ones(E, bool)
    newgrp[1:] = ckey[1:] != ckey[:-1]
    start_idx = np.where(newgrp)[0]
    starts[start_idx] = start_idx
    starts = np.maximum.accumulate(starts)
    rank = np.arange(E) - starts
    pos = group_base[gs] + rank           # slot within the core's slot space

    idxf = np.zeros((N_CORES, slots), np.int16)
    rowf = np.zeros((N_CORES, slots), np.float32)
    valf = np.zeros((N_CORES, slots), np.float32)
    cidx = np.where(gs % 2 == 0, cs, cs - HALF).astype(np.int16)
    idxf[cos, pos] = cidx
    rowf[cos, pos] = (rs % W).astype(np.float32)
    valf[cos, pos] = vs

    # chunking (shared across cores): per group, chunks of <=CHUNK_BLOCKS
    chunks = []   # (w, s, nblk, blk_base, col_base)
    blk_ptr = 0
    col_ptr = 0
    for g in range(WINDOWS * 2):
        left = int(B[g])
        while left > 0:
            nb = min(CHUNK_BLOCKS, left)
            chunks.append((g // 2, g % 2, nb, blk_ptr, col_ptr))
            blk_ptr += nb
            col_ptr += nb * 8
            left -= nb
    S_tot = col_ptr

    # idx wrapped layout per chunk: slot m -> (m%16, colbase + m//16), x8 replicated
    idx2d = np.zeros((N_CORES, 16, S_tot), np.int16)
    for (_, _, nb, bb, cb) in chunks:
        seg = idxf[:, bb * P:(bb + nb) * P]          # [8, nb*128]
        idx2d[:, :, cb:cb + nb * 8] = seg.reshape(N_CORES, nb * 8, 16).transpose(0, 2, 1)
    idx2d = np.tile(idx2d, (1, 8, 1))                # [8, 128, S_tot]

    # row/val tiles: slot (p, blk) layout
    rowt = rowf.reshape(N_CORES, total_blocks, P).transpose(0, 2, 1).copy()
    valt = valf.reshape(N_CORES, total_blocks, P).transpose(0, 2, 1).copy()
    return idx2d, rowt, valt, B, chunks, total_blocks, S_tot


def _build(B, chunks, total_blocks, S_tot):
    import concourse.bacc as bacc
    import concourse.mybir as mybir
    from concourse.tile import TileContext

    nc = bacc.Bacc("TRN2", target_bir_lowering=False, debug=False,
                   num_swdge_queues=NQ)
    f32 = mybir.dt.float32
    xlo = nc.dram_tensor("xlo", [HALF, D], f32, kind="ExternalInput")
    xhi = nc.dram_tensor("xhi", [HALF, D], f32, kind="ExternalInput")
    idxs = nc.dram_tensor("idxs", [P, S_tot], mybir.dt.int16, kind="ExternalInput")
    rowd = nc.dram_tensor("rowt", [P, total_blocks], f32, kind="ExternalInput")
    vald = nc.dram_tensor("valt", [P, total_blocks], f32, kind="ExternalInput")
    out = nc.dram_tensor("out", [ROWS_PER_CORE, D], f32, kind="ExternalOutput")

    xsrc = (xlo, xhi)
    # last-block flag per window for matmul stop
    win_blocks = [int(B[2 * w] + B[2 * w + 1]) for w in range(WINDOWS)]

    with TileContext(nc) as tc:
        with (
            tc.tile_pool(name="meta", bufs=1) as meta,
            tc.tile_pool(name="gat", bufs=6) as gat,
            tc.tile_pool(name="selp", bufs=8) as selp,
            tc.tile_pool(name="psum", bufs=8, space="PSUM") as psp,
            tc.tile_pool(name="ost", bufs=2) as ostp,
        ):
            idx_tile = meta.tile([P, S_tot], mybir.dt.int16)
            nc.sync.dma_start(out=idx_tile[:], in_=idxs[:, :])
            row_tile = meta.tile([P, total_blocks], f32)
            nc.sync.dma_start(out=row_tile[:], in_=rowd[:, :])
            val_tile = meta.tile([P, total_blocks], f32)
            nc.sync.dma_start(out=val_tile[:], in_=vald[:, :])
            iota_i = meta.tile([P, W], mybir.dt.int32)
            nc.gpsimd.iota(iota_i[:], pattern=[[1, W]], base=0, channel_multiplier=0)
            iota_f = meta.tile([P, W], f32)
            nc.vector.tensor_copy(out=iota_f[:], in_=iota_i[:])

            qi = 0
            ci = 0
            out_stage = None
            for w in range(WINDOWS):
                psum_t = psp.tile([W, D], f32)
                nwin = win_blocks[w]
                done = 0
                while done < nwin:
                    cw, cs_, nb, bb, cb = chunks[ci]
                    assert cw == w
                    ci += 1
                    g = gat.tile([P, CHUNK_BLOCKS * D], f32, tag="g")
                    nc.gpsimd.dma_gather(
                        out_ap=g[:, :nb * D].rearrange("p (k d) -> p k d", d=D),
                        in_ap=xsrc[cs_][:],
                        idxs_ap=idx_tile[:, cb:cb + nb * 8],
                        num_idxs=nb * P,
                        num_idxs_reg=nb * P,
                        elem_size=D,
                        queue_num=qi % NQ,
                    )
                    qi += 1
                    for j in range(nb):
                        b = bb + j
                        selt = selp.tile([P, W], f32, tag="sel")
                        nc.vector.tensor_scalar(
                            out=selt[:], in0=iota_f[:],
                            scalar1=row_tile[:, b:b + 1],
                            scalar2=val_tile[:, b:b + 1],
                            op0=mybir.AluOpType.is_equal,
                            op1=mybir.AluOpType.mult,
                        )
                        nc.tensor.matmul(
                            out=psum_t[:, :],
                            lhsT=selt[:],
                            rhs=g[:, j * D:(j + 1) * D],
                            start=(done + j == 0),
                            stop=(done + j == nwin - 1),
                        )
                    done += nb
                wi = w % OUT_BATCH
                if wi == 0:
                    out_stage = ostp.tile([W, OUT_BATCH * D], f32)
                nc.scalar.copy(out=out_stage[:, wi * D:(wi + 1) * D], in_=psum_t[:, :])
                if wi == OUT_BATCH - 1:
                    w0 = w - (OUT_BATCH - 1)
                    dview = out[w0 * W:(w + 1) * W, :].rearrange(
                        "(g p) f -> p g f", p=W)
                    sview = out_stage[:].rearrange("p (g f) -> p g f", f=D)
                    nc.sync.dma_start(out=dview, in_=sview)
    nc.compile()
    return nc


def kernel(x, row, col, val, idx):
    global LAST_EXEC_NS
    from concourse.bass_utils import run_bass_kernel_spmd

    x = np.ascontiguousarray(np.asarray(x), dtype=np.float32)
    row = np.asarray(row).astype(np.int64)
    col = np.asarray(col).astype(np.int64)
    val = np.ascontiguousarray(np.asarray(val), dtype=np.float32)

    idx2d, rowt, valt, B, chunks, total_blocks, S_tot = _pack(row, col, val)
    nc = _build(B, chunks, total_blocks, S_tot)

    xlo = np.ascontiguousarray(x[:HALF])
    xhi = np.ascontiguousarray(x[HALF:])
    in_maps = [
        {"xlo": xlo, "xhi": xhi, "idxs": idx2d[c], "rowt": rowt[c],
         "valt": valt[c]}
        for c in range(N_CORES)
    ]
    trace = os.environ.get("BASS_KERNEL_TRACE", "0") == "1"
    res = run_bass_kernel_spmd(nc, in_maps, list(range(N_CORES)), trace=trace)
    LAST_EXEC_NS = res.exec_time_ns
    outs = [np.asarray(res.results[c]["out"]) for c in range(N_CORES)]
    return np.concatenate(outs, axis=0)


# revision 8
# speedup vs baseline: 1.5469x; 1.5469x over previous
"""SpMM (COO segment-sum) kernel for trn2, 8 NeuronCores.

out[i] = sum_{e: row[e]==i} val[e] * x[col[e]]   (N=65536, E~1M, D=64)

Strategy (dest-row 1D sharding, per spec hint):
- Host: stable-sort edges by destination row; shard rows 8192/core; bucket
  edges into 64-row windows, split into two column streams (col<32768 /
  col>=32768 so node indices fit dma_gather's int16); pad each bucket to
  whole 128-edge blocks (pad: idx=0, val=0). Block counts are maxed across
  cores so all 8 cores run one SPMD program. Windows are processed in
  batches of 8; each batch's blocks are gathered in up-to-1024-index
  dma_gather chunks rotating over 4 SWDGE queues.
- Device, per chunk (<=8 blocks):
    g[p, k, :]  = x[colidx[p + 128 k]]               (dma_gather)
    eq[p, kW+r] = (iota_r == row_local[p, k])        (DVE, batched)
    sel         = eq * val[p, k]                     (DVE, batched)
  per block j:  psum_w[r, f] += sum_p sel[p, jW+r] * g[p, j, f]  (PE)
  PSUM windows drain via ACT copy to SBUF, batched DMA to out.
"""

import os
import numpy as np

N_NODES = 65536
D = 64
P = 128
N_CORES = 8
ROWS_PER_CORE = N_NODES // N_CORES   # 8192
W = 64                               # rows per PSUM window
WINDOWS = ROWS_PER_CORE // W         # 128
G_W = 8                              # windows per batch (PSUM live set)
HALF = N_NODES // 2                  # int16-addressable half
CHUNK_BLOCKS = 8                     # <=1024 idxs per dma_gather
NQ = 4                               # SWDGE queues
OUT_BATCH = 16                       # windows per output DMA

LAST_EXEC_NS = None


def _o_index(w, s):
    return (w // G_W) * (2 * G_W) + s * G_W + (w % G_W)


def _pack(row, col, val):
    """Host-side packing. Returns per-core device arrays + shared program map."""
    E = row.shape[0]
    core = row // ROWS_PER_CORE
    win = (row % ROWS_PER_CORE) // W
    strm = (col >= HALF).astype(np.int64)
    NG = WINDOWS * 2

    gkey = _o_index(win, strm)
    order = np.lexsort((row, gkey, core))
    rs, cs, vs, gs, cos = (row[order], col[order], val[order],
                           gkey[order], core[order])

    cnt = np.zeros((N_CORES, NG), np.int64)
    np.add.at(cnt, (cos, gs), 1)
    B = -(-cnt // P).max(axis=0) * -1       # ceil then max: see below
    B = (-(-cnt // P)).max(axis=0)          # [NG] blocks per group, shared
    # every window needs >=1 block so its PSUM window is written
    for w in range(WINDOWS):
        oL, oH = _o_index(w, 0), _o_index(w, 1)
        if B[oL] + B[oH] == 0:
            B[oL] = 1
    group_base = np.zeros(NG + 1, np.int64)
    np.cumsum(B * P, out=group_base[1:])
    total_blocks = int(B.sum())
    slots = total_blocks * P

    # per-edge slot position
    ckey = cos * NG + gs
    starts = np.zeros(E, np.int64)
    newgrp = np.ones(E, bool)
    newgrp[1:] = ckey[1:] != ckey[:-1]
    start_idx = np.where(newgrp)[0]
    starts[start_idx] = start_idx
    starts = np.maximum.accumulate(starts)
    rank = np.arange(E) - starts
    pos = group_base[gs] + rank

    idxf = np.zeros((N_CORES, slots), np.int16)
    rowf = np.zeros((N_CORES, slots), np.float32)
    valf = np.zeros((N_CORES, slots), np.float32)
    cidx = np.where(gs % 16 < G_W, cs, cs - HALF).astype(np.int16)
    idxf[cos, pos] = cidx
    rowf[cos, pos] = (rs % W).astype(np.float32)
    valf[cos, pos] = vs

    # block -> (window, stream) map in slot order
    blk_w = np.zeros(total_blocks, np.int64)
    blk_s = np.zeros(total_blocks, np.int64)
    bp = 0
    group_of_o = []
    for o in range(NG):
        b_ = o % (2 * G_W)
        s_ = b_ // G_W
        w_ = (o // (2 * G_W)) * G_W + (b_ % G_W)
        group_of_o.append((w_, s_))
        blk_w[bp:bp + B[o]] = w_
        blk_s[bp:bp + B[o]] = s_
        bp += B[o]

    # chunks: consecutive blocks of one (batch, stream) section, <=CHUNK_BLOCKS
    chunks = []   # (bat, s, blk_base, nblk, col_base)
    col_ptr = 0
    bp = 0
    for bat in range(WINDOWS // G_W):
        for s_ in range(2):
            o0 = bat * 2 * G_W + s_ * G_W
            sec_blocks = int(B[o0:o0 + G_W].sum())
            done = 0
            while done < sec_blocks:
                nb = min(CHUNK_BLOCKS, sec_blocks - done)
                chunks.append((bat, s_, bp, nb, col_ptr))
                col_ptr += nb * 8
                bp += nb
                done += nb
    S_tot = col_ptr

    # first/last block per window (for matmul start/stop)
    win_first = np.full(WINDOWS, -1, np.int64)
    win_last = np.zeros(WINDOWS, np.int64)
    for b in range(total_blocks):
        w_ = blk_w[b]
        if win_first[w_] < 0:
            win_first[w_] = b
        win_last[w_] = b

    # idx wrapped layout per chunk, x8 replicated
    idx2d = np.zeros((N_CORES, 16, S_tot), np.int16)
    for (_, s_, bb, nb, cb) in chunks:
        seg = idxf[:, bb * P:(bb + nb) * P]
        idx2d[:, :, cb:cb + nb * 8] = seg.reshape(N_CORES, nb * 8, 16).transpose(0, 2, 1)
    idx2d = np.tile(idx2d, (1, 8, 1))

    rowt = rowf.reshape(N_CORES, total_blocks, P).transpose(0, 2, 1).copy()
    valt = valf.reshape(N_CORES, total_blocks, P).transpose(0, 2, 1).copy()
    return (idx2d, rowt, valt, chunks, blk_w, win_first, win_last,
            total_blocks, S_tot)


def _build(chunks, blk_w, win_first, win_last, total_blocks, S_tot):
    import concourse.bacc as bacc
    import concourse.mybir as mybir
    from concourse.tile import TileContext

    nc = bacc.Bacc("TRN2", target_bir_lowering=False, debug=False,
                   num_swdge_queues=NQ)
    f32 = mybir.dt.float32
    xlo = nc.dram_tensor("xlo", [HALF, D], f32, kind="ExternalInput")
    xhi = nc.dram_tensor("xhi", [HALF, D], f32, kind="ExternalInput")
    idxs = nc.dram_tensor("idxs", [P, S_tot], mybir.dt.int16, kind="ExternalInput")
    rowd = nc.dram_tensor("rowt", [P, total_blocks], f32, kind="ExternalInput")
    vald = nc.dram_tensor("valt", [P, total_blocks], f32, kind="ExternalInput")
    out = nc.dram_tensor("out", [ROWS_PER_CORE, D], f32, kind="ExternalOutput")
    xsrc = (xlo, xhi)

    with TileContext(nc) as tc:
        with (
            tc.tile_pool(name="meta", bufs=1) as meta,
            tc.tile_pool(name="gat", bufs=6) as gat,
            tc.tile_pool(name="selp", bufs=6) as selp,
            tc.tile_pool(name="psum", bufs=8, space="PSUM") as psp,
            tc.tile_pool(name="ost", bufs=2) as ostp,
        ):
            idx_tile = meta.tile([P, S_tot], mybir.dt.int16)
            nc.sync.dma_start(out=idx_tile[:], in_=idxs[:, :])
            row_tile = meta.tile([P, total_blocks], f32)
            nc.sync.dma_start(out=row_tile[:], in_=rowd[:, :])
            val_tile = meta.tile([P, total_blocks], f32)
            nc.sync.dma_start(out=val_tile[:], in_=vald[:, :])
            iota_i = meta.tile([P, CHUNK_BLOCKS * W], mybir.dt.int32)
            nc.gpsimd.iota(iota_i[:], pattern=[[0, CHUNK_BLOCKS], [1, W]],
                           base=0, channel_multiplier=0)
            iota_f = meta.tile([P, CHUNK_BLOCKS * W], f32)
            nc.vector.tensor_copy(out=iota_f[:], in_=iota_i[:])

            def drain_batch(bat):
                nonlocal out_stage
                for w_ in range(bat * G_W, (bat + 1) * G_W):
                    wi = w_ % OUT_BATCH
                    if wi == 0:
                        out_stage = ostp.tile([W, OUT_BATCH * D], f32)
                    nc.scalar.copy(out=out_stage[:, wi * D:(wi + 1) * D],
                                   in_=psum_of.pop(w_)[:, :])
                    if wi == OUT_BATCH - 1:
                        w0 = w_ - (OUT_BATCH - 1)
                        dview = out[w0 * W:(w_ + 1) * W, :].rearrange(
                            "(g p) f -> p g f", p=W)
                        sview = out_stage[:].rearrange("p (g f) -> p g f", f=D)
                        nc.sync.dma_start(out=dview, in_=sview)

            psum_of = {}
            out_stage = None
            cur_bat = 0
            qi = 0
            for (bat, s_, bb, nb, cb) in chunks:
                if bat != cur_bat:
                    drain_batch(cur_bat)
                    cur_bat = bat
                g = gat.tile([P, CHUNK_BLOCKS * D], f32, tag="g")
                nc.gpsimd.dma_gather(
                    out_ap=g[:, :nb * D].rearrange("p (k d) -> p k d", d=D),
                    in_ap=xsrc[s_][:],
                    idxs_ap=idx_tile[:, cb:cb + nb * 8],
                    num_idxs=nb * P,
                    num_idxs_reg=nb * P,
                    elem_size=D,
                    queue_num=qi % NQ,
                )
                qi += 1

                selt = selp.tile([P, CHUNK_BLOCKS * W], f32, tag="sel")
                sel3 = selt[:, :nb * W].rearrange("p (k w) -> p k w", w=W)
                nc.vector.tensor_tensor(
                    out=sel3,
                    in0=iota_f[:, :nb * W].rearrange("p (k w) -> p k w", w=W),
                    in1=row_tile[:, bb:bb + nb].to_broadcast([P, nb, W]),
                    op=mybir.AluOpType.is_equal,
                )
                nc.vector.tensor_tensor(
                    out=sel3,
                    in0=sel3,
                    in1=val_tile[:, bb:bb + nb].to_broadcast([P, nb, W]),
                    op=mybir.AluOpType.mult,
                )

                for j in range(nb):
                    b = bb + j
                    w_ = int(blk_w[b])
                    if w_ not in psum_of:
                        psum_of[w_] = psp.tile([W, D], f32, name='psw', tag='psw')
                    nc.tensor.matmul(
                        out=psum_of[w_][:, :],
                        lhsT=selt[:, j * W:(j + 1) * W],
                        rhs=g[:, j * D:(j + 1) * D],
                        start=(b == win_first[w_]),
                        stop=(b == win_last[w_]),
                    )
            drain_batch(cur_bat)
    nc.compile()
    return nc


def kernel(x, row, col, val, idx):
    global LAST_EXEC_NS
    from concourse.bass_utils import run_bass_kernel_spmd

    x = np.ascontiguousarray(np.asarray(x), dtype=np.float32)
    row = np.asarray(row).astype(np.int64)
    col = np.asarray(col).astype(np.int64)
    val = np.ascontiguousarray(np.asarray(val), dtype=np.float32)

    (idx2d, rowt, valt, chunks, blk_w, win_first, win_last,
     total_blocks, S_tot) = _pack(row, col, val)
    nc = _build(chunks, blk_w, win_first, win_last, total_blocks, S_tot)

    xlo = np.ascontiguousarray(x[:HALF])
    xhi = np.ascontiguousarray(x[HALF:])
    in_maps = [
        {"xlo": xlo, "xhi": xhi, "idxs": idx2d[c], "rowt": rowt[c],
         "valt": valt[c]}
        for c in range(N_CORES)
    ]
    trace = os.environ.get("BASS_KERNEL_TRACE", "0") == "1"
    res = run_bass_kernel_spmd(nc, in_maps, list(range(N_CORES)), trace=trace)
    LAST_EXEC_NS = res.exec_time_ns
    outs = [np.asarray(res.results[c]["out"]) for c in range(N_CORES)]
    return np.concatenate(outs, axis=0)


# revision 10
# speedup vs baseline: 1.7452x; 1.1282x over previous
"""SpMM (COO segment-sum) kernel for trn2, 8 NeuronCores.

out[i] = sum_{e: row[e]==i} val[e] * x[col[e]]   (N=65536, E~1M, D=64)

Strategy (dest-row 1D sharding, per spec hint):
- Host: stable-sort edges by destination row; shard rows 8192/core; bucket
  edges into 64-row windows, split into two column streams (col<32768 /
  col>=32768 so node indices fit dma_gather's int16); pad each bucket to
  whole 128-edge blocks (pad: idx=0, val=0). Block counts are maxed across
  cores so all 8 cores run one SPMD program. Windows are processed in
  batches of 8; each batch's blocks are gathered in up-to-1024-index
  dma_gather chunks rotating over 4 SWDGE queues.
- Device, per chunk (<=8 blocks):
    g[p, k, :]  = x[colidx[p + 128 k]]               (dma_gather)
    eq[p, kW+r] = (iota_r == row_local[p, k])        (DVE, batched)
    sel         = eq * val[p, k]                     (DVE, batched)
  per block j:  psum_w[r, f] += sum_p sel[p, jW+r] * g[p, j, f]  (PE)
  PSUM windows drain via ACT copy to SBUF, batched DMA to out.
"""

import os
import numpy as np

N_NODES = 65536
D = 64
P = 128
N_CORES = 8
ROWS_PER_CORE = N_NODES // N_CORES   # 8192
W = 64                               # rows per PSUM window
WINDOWS = ROWS_PER_CORE // W         # 128
G_W = 8                              # windows per batch (PSUM live set)
HALF = N_NODES // 2                  # int16-addressable half
CHUNK_BLOCKS = 8                     # <=1024 idxs per dma_gather
NQ = 4                               # SWDGE queues
OUT_BATCH = 16                       # windows per output DMA

LAST_EXEC_NS = None


def _o_index(w, s):
    return (w // G_W) * (2 * G_W) + s * G_W + (w % G_W)


def _pack(row, col, val):
    """Host-side packing. Returns per-core device arrays + shared program map."""
    E = row.shape[0]
    core = row // ROWS_PER_CORE
    win = (row % ROWS_PER_CORE) // W
    strm = (col >= HALF).astype(np.int64)
    NG = WINDOWS * 2

    gkey = _o_index(win, strm)
    order = np.lexsort((row, gkey, core))
    rs, cs, vs, gs, cos = (row[order], col[order], val[order],
                           gkey[order], core[order])

    cnt = np.zeros((N_CORES, NG), np.int64)
    np.add.at(cnt, (cos, gs), 1)
    B = -(-cnt // P).max(axis=0) * -1       # ceil then max: see below
    B = (-(-cnt // P)).max(axis=0)          # [NG] blocks per group, shared
    # every window needs >=1 block so its PSUM window is written
    for w in range(WINDOWS):
        oL, oH = _o_index(w, 0), _o_index(w, 1)
        if B[oL] + B[oH] == 0:
            B[oL] = 1
    group_base = np.zeros(NG + 1, np.int64)
    np.cumsum(B * P, out=group_base[1:])
    total_blocks = int(B.sum())
    slots = total_blocks * P

    # per-edge slot position
    ckey = cos * NG + gs
    starts = np.zeros(E, np.int64)
    newgrp = np.ones(E, bool)
    newgrp[1:] = ckey[1:] != ckey[:-1]
    start_idx = np.where(newgrp)[0]
    starts[start_idx] = start_idx
    starts = np.maximum.accumulate(starts)
    rank = np.arange(E) - starts
    pos = group_base[gs] + rank

    idxf = np.zeros((N_CORES, slots), np.int16)
    rowf = np.zeros((N_CORES, slots), np.float32)
    valf = np.zeros((N_CORES, slots), np.float32)
    cidx = np.where(gs % 16 < G_W, cs, cs - HALF).astype(np.int16)
    idxf[cos, pos] = cidx
    rowf[cos, pos] = (rs % W).astype(np.float32)
    valf[cos, pos] = vs

    # block -> (window, stream) map in slot order
    blk_w = np.zeros(total_blocks, np.int64)
    blk_s = np.zeros(total_blocks, np.int64)
    bp = 0
    group_of_o = []
    for o in range(NG):
        b_ = o % (2 * G_W)
        s_ = b_ // G_W
        w_ = (o // (2 * G_W)) * G_W + (b_ % G_W)
        group_of_o.append((w_, s_))
        blk_w[bp:bp + B[o]] = w_
        blk_s[bp:bp + B[o]] = s_
        bp += B[o]

    # chunks: consecutive blocks of one (batch, stream) section, <=CHUNK_BLOCKS
    chunks = []   # (bat, s, blk_base, nblk, col_base)
    col_ptr = 0
    bp = 0
    for bat in range(WINDOWS // G_W):
        for s_ in range(2):
            o0 = bat * 2 * G_W + s_ * G_W
            sec_blocks = int(B[o0:o0 + G_W].sum())
            done = 0
            while done < sec_blocks:
                nb = min(CHUNK_BLOCKS, sec_blocks - done)
                chunks.append((bat, s_, bp, nb, col_ptr))
                col_ptr += nb * 8
                bp += nb
                done += nb
    S_tot = col_ptr

    # first/last block per window (for matmul start/stop)
    win_first = np.full(WINDOWS, -1, np.int64)
    win_last = np.zeros(WINDOWS, np.int64)
    for b in range(total_blocks):
        w_ = blk_w[b]
        if win_first[w_] < 0:
            win_first[w_] = b
        win_last[w_] = b

    # idx wrapped layout per chunk, x8 replicated
    idx2d = np.zeros((N_CORES, 16, S_tot), np.int16)
    for (_, s_, bb, nb, cb) in chunks:
        seg = idxf[:, bb * P:(bb + nb) * P]
        idx2d[:, :, cb:cb + nb * 8] = seg.reshape(N_CORES, nb * 8, 16).transpose(0, 2, 1)
    idx2d = np.tile(idx2d, (1, 8, 1))

    rowt = rowf.reshape(N_CORES, total_blocks, P).transpose(0, 2, 1).copy()
    valt = valf.reshape(N_CORES, total_blocks, P).transpose(0, 2, 1).copy()
    return (idx2d, rowt, valt, chunks, blk_w, win_first, win_last,
            total_blocks, S_tot)


def _build(chunks, blk_w, win_first, win_last, total_blocks, S_tot):
    import concourse.bacc as bacc
    import concourse.mybir as mybir
    from concourse.tile import TileContext

    nc = bacc.Bacc("TRN2", target_bir_lowering=False, debug=False,
                   num_swdge_queues=NQ)
    f32 = mybir.dt.float32
    xlo = nc.dram_tensor("xlo", [HALF, D], f32, kind="ExternalInput")
    xhi = nc.dram_tensor("xhi", [HALF, D], f32, kind="ExternalInput")
    idxs = nc.dram_tensor("idxs", [P, S_tot], mybir.dt.int16, kind="ExternalInput")
    rowd = nc.dram_tensor("rowt", [P, total_blocks], f32, kind="ExternalInput")
    vald = nc.dram_tensor("valt", [P, total_blocks], f32, kind="ExternalInput")
    out = nc.dram_tensor("out", [ROWS_PER_CORE, D], f32, kind="ExternalOutput")
    xsrc = (xlo, xhi)

    with TileContext(nc) as tc:
        with (
            tc.tile_pool(name="meta", bufs=1) as meta,
            tc.tile_pool(name="gat", bufs=8) as gat,
            tc.tile_pool(name="selp", bufs=8) as selp,
            tc.tile_pool(name="psum", bufs=8, space="PSUM") as psp,
            tc.tile_pool(name="ost", bufs=2) as ostp,
        ):
            idx_tile = meta.tile([P, S_tot], mybir.dt.int16)
            # split the idx load so early gathers start before the whole
            # table has landed (Tile tracks sub-range deps)
            n_split = 4
            step = -(-S_tot // n_split)
            for si in range(n_split):
                a, b_ = si * step, min((si + 1) * step, S_tot)
                if a < b_:
                    nc.sync.dma_start(out=idx_tile[:, a:b_], in_=idxs[:, a:b_])
            row_tile = meta.tile([P, total_blocks], f32)
            nc.sync.dma_start(out=row_tile[:], in_=rowd[:, :])
            val_tile = meta.tile([P, total_blocks], f32)
            nc.sync.dma_start(out=val_tile[:], in_=vald[:, :])
            iota_i = meta.tile([P, CHUNK_BLOCKS * W], mybir.dt.int32)
            nc.gpsimd.iota(iota_i[:], pattern=[[0, CHUNK_BLOCKS], [1, W]],
                           base=0, channel_multiplier=0)
            iota_f = meta.tile([P, CHUNK_BLOCKS * W], f32)
            nc.vector.tensor_copy(out=iota_f[:], in_=iota_i[:])

            def drain_batch(bat):
                nonlocal out_stage
                for w_ in range(bat * G_W, (bat + 1) * G_W):
                    wi = w_ % OUT_BATCH
                    if wi == 0:
                        out_stage = ostp.tile([W, OUT_BATCH * D], f32)
                    nc.scalar.copy(out=out_stage[:, wi * D:(wi + 1) * D],
                                   in_=psum_of.pop(w_)[:, :])
                    if wi == OUT_BATCH - 1:
                        w0 = w_ - (OUT_BATCH - 1)
                        dview = out[w0 * W:(w_ + 1) * W, :].rearrange(
                            "(g p) f -> p g f", p=W)
                        sview = out_stage[:].rearrange("p (g f) -> p g f", f=D)
                        nc.sync.dma_start(out=dview, in_=sview)

            psum_of = {}
            out_stage = None
            cur_bat = 0
            qi = 0
            for (bat, s_, bb, nb, cb) in chunks:
                if bat != cur_bat:
                    drain_batch(cur_bat)
                    cur_bat = bat
                g = gat.tile([P, CHUNK_BLOCKS * D], f32, tag="g")
                nc.gpsimd.dma_gather(
                    out_ap=g[:, :nb * D].rearrange("p (k d) -> p k d", d=D),
                    in_ap=xsrc[s_][:],
                    idxs_ap=idx_tile[:, cb:cb + nb * 8],
                    num_idxs=nb * P,
                    num_idxs_reg=nb * P,
                    elem_size=D,
                    queue_num=qi % NQ,
                )
                qi += 1

                selt = selp.tile([P, CHUNK_BLOCKS * W], f32, tag="sel")
                sel3 = selt[:, :nb * W].rearrange("p (k w) -> p k w", w=W)
                nc.vector.tensor_tensor(
                    out=sel3,
                    in0=iota_f[:, :nb * W].rearrange("p (k w) -> p k w", w=W),
                    in1=row_tile[:, bb:bb + nb].to_broadcast([P, nb, W]),
                    op=mybir.AluOpType.is_equal,
                )
                nc.vector.tensor_tensor(
                    out=sel3,
                    in0=sel3,
                    in1=val_tile[:, bb:bb + nb].to_broadcast([P, nb, W]),
                    op=mybir.AluOpType.mult,
                )

                for j in range(nb):
                    b = bb + j
                    w_ = int(blk_w[b])
                    if w_ not in psum_of:
                        psum_of[w_] = psp.tile([W, D], f32, name='psw', tag='psw')
                    nc.tensor.matmul(
                        out=psum_of[w_][:, :],
                        lhsT=selt[:, j * W:(j + 1) * W],
                        rhs=g[:, j * D:(j + 1) * D],
                        start=(b == win_first[w_]),
                        stop=(b == win_last[w_]),
                    )
            drain_batch(cur_bat)
    nc.compile()
    return nc


def kernel(x, row, col, val, idx):
    global LAST_EXEC_NS
    from concourse.bass_utils import run_bass_kernel_spmd

    x = np.ascontiguousarray(np.asarray(x), dtype=np.float32)
    row = np.asarray(row).astype(np.int64)
    col = np.asarray(col).astype(np.int64)
    val = np.ascontiguousarray(np.asarray(val), dtype=np.float32)

    (idx2d, rowt, valt, chunks, blk_w, win_first, win_last,
     total_blocks, S_tot) = _pack(row, col, val)
    nc = _build(chunks, blk_w, win_first, win_last, total_blocks, S_tot)

    xlo = np.ascontiguousarray(x[:HALF])
    xhi = np.ascontiguousarray(x[HALF:])
    in_maps = [
        {"xlo": xlo, "xhi": xhi, "idxs": idx2d[c], "rowt": rowt[c],
         "valt": valt[c]}
        for c in range(N_CORES)
    ]
    trace = os.environ.get("BASS_KERNEL_TRACE", "0") == "1"
    res = run_bass_kernel_spmd(nc, in_maps, list(range(N_CORES)), trace=trace)
    LAST_EXEC_NS = res.exec_time_ns
    outs = [np.asarray(res.results[c]["out"]) for c in range(N_CORES)]
    return np.concatenate(outs, axis=0)
